# revision 3
# baseline (speedup 1.0000x reference)
"""Trainium2 Bass kernel for nn_AttentionAggregate_Weight (gnn_message_passing).

Computes, per node n with K=32 neighbors and D=128 features:
    score[n,k] = tanh(nodes_key[n].v1 + middle_key[n,k].v2 + a_b)
    out[n,:]   = sum_k softmax_k(score)[n,k] * middle_value[n,k,:]
where v1 = W1.T @ a_w and v2 = W2.T @ a_w are folded on the host (the
reference's p1/p2 projections only ever appear dotted with a_w; tanh
outputs lie in [-1,1] so the softmax needs no max subtraction and the
denominator folds into one final per-node scale).

Distribution: pure data parallel over the node axis across 8 NeuronCores
(2500 nodes each); the tiny folded params are replicated. On-core layout
puts 125 nodes on partitions and (k, d) on the free axis. All compute is
per-k fused DVE ops (scalar_tensor_tensor multiply+row-sum for scores,
multiply+accumulate chains for the weighted values) — small free dims
dodge the DVE's per-op pipeline-DRAIN penalty — with ACT doing tanh and
a fused exp+row-sum. The tile loop is software-pipelined two tiles deep
(scores of tile t emitted before values of tile t-2) so the in-order
engines never stall on the cross-engine score->coefficient chain, and
the big key/value loads stream on the HWDGE ring at the HBM roofline.

Self-contained: hardcodes shapes/sharding; no file I/O.
"""

from contextlib import ExitStack

import numpy as np

N, K, D = 20000, 32, 128
KD = K * D
NCONST = KD + D + 1
N_CORES = 8
NPC = N // N_CORES
P = 125  # nodes per tile (partition dim)
SKEW = 2
BUFS = 3


# ---------------------------------------------------------------------------
# Wait legalization: this walrus build accepts at most ONE semaphore wait per
# instruction; split extras onto same-engine Drain carriers at the BIR level.
# ---------------------------------------------------------------------------
def _legalize_bir_waits(bir_bytes: bytes) -> bytes:
    import orjson

    m = orjson.loads(bir_bytes)
    n = 0
    for f in m.get("functions", []):
        for b in f.get("blocks", []):
            insts = b.get("instructions") or []
            out = []
            changed = False
            for ins in insts:
                si = ins.get("sync_info")
                waits = (si or {}).get("on_wait") or []
                if len(waits) > 1:
                    changed = True
                    for w in waits[:-1]:
                        n += 1
                        out.append(
                            {
                                "debug": ins.get("debug", 0),
                                "engine": ins.get("engine"),
                                "ins": [],
                                "outs": [],
                                "name": f"I-wfix-{n}",
                                "opcode": "Drain",
                                "sync_info": {"on_update": [], "on_wait": [w]},
                            }
                        )
                    si["on_wait"] = [waits[-1]]
                out.append(ins)
            if changed:
                b["instructions"] = out
    return orjson.dumps(m)


_waitfix_installed = False


def _install_waitfix():
    global _waitfix_installed
    if _waitfix_installed:
        return
    import concourse.bass as bass

    orig = bass.Bass.to_json_bytes

    def patched(self):
        return _legalize_bir_waits(orig(self))

    bass.Bass.to_json_bytes = patched
    _waitfix_installed = True


# ---------------------------------------------------------------------------
# Kernel builder (per-core: NPC nodes)
# ---------------------------------------------------------------------------
def _build_kernel():
    import concourse.bass as bass
    import concourse.tile as tile
    from concourse import mybir

    f32 = mybir.dt.float32
    n_tiles = NPC // P

    nc = bass.Bass()
    mk = nc.dram_tensor("mk", (NPC, K, D), f32, kind="ExternalInput")
    nk = nc.dram_tensor("nk", (NPC, D), f32, kind="ExternalInput")
    mv = nc.dram_tensor("mv", (NPC, K, D), f32, kind="ExternalInput")
    consts = nc.dram_tensor("consts", (128, NCONST), f32, kind="ExternalInput")
    out = nc.dram_tensor("out", (NPC, D), f32, kind="ExternalOutput")

    with tile.TileContext(nc) as tc, ExitStack() as ctx:
        singles = ctx.enter_context(tc.tile_pool(name="singles", bufs=1))
        keys = ctx.enter_context(tc.tile_pool(name="keys", bufs=BUFS + SKEW))
        vals = ctx.enter_context(tc.tile_pool(name="vals", bufs=BUFS + SKEW))
        nks = ctx.enter_context(tc.tile_pool(name="nks", bufs=BUFS))
        outs = ctx.enter_context(tc.tile_pool(name="outs", bufs=BUFS))
        smalls = ctx.enter_context(tc.tile_pool(name="smalls", bufs=BUFS + SKEW))
        junks = ctx.enter_context(tc.tile_pool(name="junks", bufs=2))

        ct = singles.tile([128, NCONST], f32)
        nc.gpsimd.dma_start(out=ct, in_=consts[:])
        v1_sb = ct[0:P, KD : KD + D]
        ab_sb = ct[0:P, KD + D : KD + D + 1]
        v2row = ct[0:P, 0:D]
        # dummy touch: DVE observes the const-DMA semaphore before the loop
        dum = singles.tile([1, 1], f32)
        nc.vector.tensor_copy(out=dum, in_=ct[0:1, 0:1])

        def emit_loads(t):
            rows = slice(t * P, (t + 1) * P)
            key3 = keys.tile([P, K, D], f32, tag="key3", name=f"key3_{t}")
            nc.sync.dma_start(out=key3, in_=mk[rows])
            val3 = vals.tile([P, K, D], f32, tag="val3", name=f"val3_{t}")
            nc.sync.dma_start(out=val3, in_=mv[rows])
            nk_t = nks.tile([P, D], f32, tag="nk_t", name=f"nk_{t}")
            nc.gpsimd.dma_start(out=nk_t, in_=nk[rows])
            return {"key3": key3, "val3": val3, "nk_t": nk_t}

        def emit_scores(t, h):
            key3, nk_t = h["key3"], h["nk_t"]
            junk = junks.tile([P, D], f32, tag="junk", name=f"junk_{t}")
            s1b = smalls.tile([P, 1], f32, tag="s1b", name=f"s1b_{t}")
            sc_raw = smalls.tile([P, K], f32, tag="sc_raw", name=f"scr_{t}")
            # s1 = a_b + nk.v1 — fused multiply + row-sum
            nc.vector.scalar_tensor_tensor(
                out=junk, in0=nk_t, scalar=1.0, in1=v1_sb,
                op0=mybir.AluOpType.bypass, op1=mybir.AluOpType.mult,
                accum_out=s1b,
            )
            nc.vector.tensor_add(out=s1b, in0=s1b, in1=ab_sb)
            # s2[n,k] = key[n,k].v2 — one fused multiply+row-sum per k
            for k in range(K):
                nc.vector.scalar_tensor_tensor(
                    out=junk, in0=key3[:, k, :], scalar=1.0, in1=v2row,
                    op0=mybir.AluOpType.bypass, op1=mybir.AluOpType.mult,
                    accum_out=sc_raw[:, k : k + 1],
                )
            sc = smalls.tile([P, K], f32, tag="sc", name=f"sc_{t}")
            nc.scalar.activation(
                out=sc, in_=sc_raw, func=mybir.ActivationFunctionType.Tanh,
                bias=s1b, scale=1.0,
            )
            e_t = smalls.tile([P, K], f32, tag="e_t", name=f"e_{t}")
            sums = smalls.tile([P, 1], f32, tag="sums", name=f"sums_{t}")
            nc.scalar.activation(
                out=e_t, in_=sc, func=mybir.ActivationFunctionType.Exp,
                accum_out=sums,
            )
            recip = smalls.tile([P, 1], f32, tag="recip", name=f"recip_{t}")
            nc.vector.reciprocal(out=recip, in_=sums)
            h["e_t"], h["recip"] = e_t, recip

        def emit_values(t, h):
            val3, e_t, recip = h["val3"], h["e_t"], h["recip"]
            rows = slice(t * P, (t + 1) * P)
            out_t = outs.tile([P, D], f32, tag="out_t", name=f"out_{t}")
            # out_t = sum_k val_k * e_k via fused multiply-accumulate chain
            nc.vector.tensor_scalar_mul(
                out=out_t, in0=val3[:, 0, :], scalar1=e_t[:, 0:1]
            )
            for k in range(1, K):
                nc.vector.scalar_tensor_tensor(
                    out=out_t, in0=val3[:, k, :], scalar=e_t[:, k : k + 1],
                    in1=out_t,
                    op0=mybir.AluOpType.mult, op1=mybir.AluOpType.add,
                )
            nc.vector.tensor_scalar_mul(out=out_t, in0=out_t, scalar1=recip)
            nc.gpsimd.dma_start(out=out[rows], in_=out_t)

        handles = {}
        for i in range(n_tiles + SKEW):
            if i < n_tiles:
                h = emit_loads(i)
                emit_scores(i, h)
                handles[i] = h
            j = i - SKEW
            if j >= 0:
                emit_values(j, handles.pop(j))

    return nc


_nc_cache = {}


def _get_nc():
    if "main" not in _nc_cache:
        _install_waitfix()
        nc = _build_kernel()
        nc.finalize()
        _nc_cache["main"] = nc
    return _nc_cache["main"]


def kernel(middle_key, nodes_key, middle_value, W1, W2, a_w, a_b):
    middle_key = np.ascontiguousarray(middle_key, np.float32)
    nodes_key = np.ascontiguousarray(nodes_key, np.float32)
    middle_value = np.ascontiguousarray(middle_value, np.float32)

    v1 = (W1.astype(np.float64).T @ a_w.astype(np.float64)).astype(np.float32)
    v2 = (W2.astype(np.float64).T @ a_w.astype(np.float64)).astype(np.float32)
    row = np.concatenate([np.tile(v2, K), v1, np.float32(a_b[:1])]).astype(np.float32)
    consts = np.ascontiguousarray(np.tile(row[None, :], (128, 1)), np.float32)

    nc = _get_nc()

    in_maps = []
    for c in range(N_CORES):
        s = slice(c * NPC, (c + 1) * NPC)
        in_maps.append(
            {
                "mk": middle_key[s],
                "nk": nodes_key[s],
                "mv": middle_value[s],
                "consts": consts,
            }
        )

    from concourse import bass2jax

    results = bass2jax.run_bass_via_pjrt(nc, in_maps, n_cores=N_CORES)
    return np.concatenate([r["out"] for r in results], axis=0).astype(np.float32)
